# revision 11
# baseline (speedup 1.0000x reference)
"""Embedding lookup kernel for Trainium2 (8 NeuronCores, SPMD).

Strategy: token-parallel gather (an embedding lookup IS a row gather:
out[b, s, :] = weight[x[b, s], :]).

- Flatten x [2, 4096] -> [8192] tokens; each of the 8 cores handles 1024
  contiguous tokens. Each core gets the FULL weight table in its DRAM.
- Per core: 4 dma_gather ops of 256 rows each, one per SWDGE queue 0-3.
  HW-verified queue semantics: queue q's descriptor generation runs on
  Q7 core pair q (cores 2q, 2q+1), whose ucode reads its indices from
  SBUF partitions [32q, 32q+32). Ops on queues 1-3 dispatch
  asynchronously (~70ns of Pool-engine time each); only queue 0 blocks
  the NX for the op's duration. Issuing q1, q2, q3 then q0 makes all
  FOUR pairs generate descriptors concurrently (~2.3us each at
  ~9ns/row) instead of the ~11us serial InstDMACopy train of the
  8-indirect-op variant.
- The mlp Q7 library (dma_gather ucode) load is the critical-path tax:
  MODIFY_POOL_CONFIG blocks all 8 Q7 cores ~8.7us, so gathers start
  ~15us in regardless of the idx DMA (~2.4us). load_library is issued
  explicitly as the first Pool instruction to start it ASAP.
- Index layout (int16; VOCAB=32000 < 2^15 so exact): the host packs, for
  each op q, its 256 token ids wrapped as [16, 16] (idx i at partition
  i%16, column i//16) and tiles that x2 into partitions [32q, 32q+32).
  All four ops pass the same [128, 16] SBUF AP; each pair reads its own
  partition group.
- dma_gather writes row i of op q to g[i%128, 2q + i//128, :]; one HWDGE
  store per op (issued as its completion semaphore fires) moves
  [128, 2, 128] to DRAM. Host: transpose(1,0,2).reshape(1024,128)
  recovers token-major order; concatenate the 8 per-core outputs.
- The Bass entry all-engine barrier is skipped; the idx load issues from
  the Scalar engine (exits the framework preamble ~0.9us before Sync).
  No final store wait: the NEFF epilogue's engine drains block until
  the HWDGE queues are empty (verified bit-exact on HW).

No collectives. Rejected alternatives (all HW-measured): 8x
indirect_dma_start (no library tax but 1.41us/op NX-serial -> ~23.0us
total), single dma_gather on queue 0 (8.4ns/row serial on one pair ->
31.1us), DRAM->DRAM indirect (NRT_EXEC_UNIT_UNRECOVERABLE), multi-index
offset APs (HW reads one offset per partition), one-hot matmul
(compute-bound).
"""

import contextlib

import numpy as np

import concourse.bass as bass
from concourse import bacc, mybir
from concourse.bass_utils import run_bass_kernel_spmd
from concourse.library_config import mlp

N_CORES = 8
B, S = 2, 4096
VOCAB, DIM = 32000, 128
P = 128
TOKENS = B * S                      # 8192
TPC = TOKENS // N_CORES             # 1024 tokens per core
NQ = 4                              # SWDGE queues / Q7 core pairs used
RPQ = TPC // NQ                     # 256 rows per queue op
CBQ = RPQ // P                      # 2 col-blocks per op
IDX_COLS = RPQ // 16                # 16 int16 per idx partition


def build_nc():
    # Skip the Bass-constructor entry barrier (gates the first DMA behind
    # all engines' init); restore the method right after construction.
    orig_barrier = bass.Bass.all_engine_barrier
    bass.Bass.all_engine_barrier = lambda self, *a, **k: None
    try:
        nc = bacc.Bacc(
            None,
            target_bir_lowering=False,
            num_swdge_queues=NQ,
            dynamic_dma_scratch_size=65536,
        )
    finally:
        bass.Bass.all_engine_barrier = orig_barrier

    x = nc.dram_tensor("x", [P, IDX_COLS], mybir.dt.int16, kind="ExternalInput")
    w = nc.dram_tensor("weight", [VOCAB, DIM], mybir.dt.float32, kind="ExternalInput")
    out = nc.dram_tensor(
        "out", [P, NQ * CBQ, DIM], mybir.dt.float32, kind="ExternalOutput"
    )

    with contextlib.ExitStack() as ctx:
        idx_tile = ctx.enter_context(
            nc.sbuf_tensor("idx_tile", [P, IDX_COLS], mybir.dt.int16)
        )
        g = ctx.enter_context(
            nc.sbuf_tensor("g", [P, NQ * CBQ, DIM], mybir.dt.float32)
        )
        s_idx = ctx.enter_context(nc.semaphore("s_idx"))
        s_gs = [ctx.enter_context(nc.semaphore(f"s_g{q}")) for q in range(NQ)]
        s_out = ctx.enter_context(nc.semaphore("s_out"))

        # Start the ~8.7us Q7 library load first -- it is the critical path.
        nc.gpsimd.load_library(mlp)

        nc.scalar.dma_start(idx_tile[:], x[:]).then_inc(s_idx, 16)

        nc.gpsimd.wait_ge(s_idx, 16)
        # Queues 1..3 dispatch async to their pairs; queue 0 last (blocks).
        for q in [1, 2, 3, 0]:
            nc.gpsimd.dma_gather(
                g[:, q * CBQ : (q + 1) * CBQ, :],
                w[:],
                idx_tile[:],
                RPQ,
                RPQ,
                DIM,
                queue_num=q,
            ).then_inc(s_gs[q], 16)

        for q in [1, 2, 3, 0]:
            nc.sync.wait_ge(s_gs[q], 16)
            nc.sync.dma_start(
                out[:, q * CBQ : (q + 1) * CBQ, :], g[:, q * CBQ : (q + 1) * CBQ, :]
            ).then_inc(s_out, 16)
    nc.compile()
    return nc


_NC_CACHE = None


def _pack_idx(idx_1d: np.ndarray) -> np.ndarray:
    """[TPC] ints -> [128, 16] int16 tile: partitions [32q, 32q+32) hold op
    q's 256 ids wrapped [16, 16] (id i at partition i%16, col i//16), x2."""
    tile = np.empty((P, IDX_COLS), dtype=np.int16)
    for q in range(NQ):
        wrapped = idx_1d[q * RPQ : (q + 1) * RPQ].astype(np.int16)
        wrapped = wrapped.reshape(IDX_COLS, 16).T        # [16, 16]
        tile[32 * q : 32 * (q + 1), :] = np.tile(wrapped, (2, 1))
    return np.ascontiguousarray(tile)


def kernel(x: np.ndarray, weight: np.ndarray, **run_kwargs):
    global _NC_CACHE
    if _NC_CACHE is None:
        _NC_CACHE = build_nc()
    nc = _NC_CACHE

    x_flat = np.asarray(x).reshape(-1)
    w = np.ascontiguousarray(np.asarray(weight, dtype=np.float32))

    in_maps = [
        {
            "x": _pack_idx(x_flat[c * TPC : (c + 1) * TPC]),
            "weight": w,
        }
        for c in range(N_CORES)
    ]
    res = run_bass_kernel_spmd(nc, in_maps, core_ids=list(range(N_CORES)), **run_kwargs)
    # out [128, 8, 128]: token q*256 + c*128 + p at [p, 2q+c, :] ->
    # transpose(1,0,2).reshape gives token-major [1024, 128].
    parts = [
        res.results[c]["out"].transpose(1, 0, 2).reshape(TPC, DIM)
        for c in range(N_CORES)
    ]
    full = np.concatenate(parts, axis=0).reshape(B, S, DIM)
    if run_kwargs:
        return full, res
    return full


# revision 12
# speedup vs baseline: 1.0079x; 1.0079x over previous
"""Embedding lookup kernel for Trainium2 (8 NeuronCores, SPMD).

Strategy: token-parallel gather (an embedding lookup IS a row gather:
out[b, s, :] = weight[x[b, s], :]).

- Flatten x [2, 4096] -> [8192] tokens; each of the 8 cores handles 1024
  contiguous tokens. Each core gets the FULL weight table in its DRAM.
- Per core: 4 dma_gather ops of 256 rows each, one per SWDGE queue 0-3.
  HW-verified queue semantics: queue q's descriptor generation runs on
  Q7 core pair q (cores 2q, 2q+1), whose ucode reads its indices from
  SBUF partitions [32q, 32q+32). Ops on queues 1-3 dispatch
  asynchronously (~70ns of Pool-engine time each); only queue 0 blocks
  the NX for the op's duration. Issuing q1, q2, q3 then q0 makes all
  FOUR pairs generate descriptors concurrently (~2.3us each at
  ~9ns/row) instead of the ~11us serial InstDMACopy train of the
  8-indirect-op variant.
- The mlp Q7 library (dma_gather ucode) load is the critical-path tax:
  MODIFY_POOL_CONFIG blocks all 8 Q7 cores ~8.7us, so gathers start
  ~15us in regardless of the idx DMA (~2.4us). load_library is issued
  explicitly as the first Pool instruction to start it ASAP.
- Index layout (int16; VOCAB=32000 < 2^15 so exact): the host packs, for
  each op q, its 256 token ids wrapped as [16, 16] (idx i at partition
  i%16, column i//16) and tiles that x2 into partitions [32q, 32q+32).
  All four ops pass the same [128, 16] SBUF AP; each pair reads its own
  partition group.
- dma_gather writes row i of op q to g[i%128, 2q + i//128, :]; one HWDGE
  store per op (issued as its completion semaphore fires) moves
  [128, 2, 128] to DRAM. Host: transpose(1,0,2).reshape(1024,128)
  recovers token-major order; concatenate the 8 per-core outputs.
- The Bass entry all-engine barrier is skipped; the idx load issues from
  the Scalar engine (exits the framework preamble ~0.9us before Sync).
  No final store wait: the NEFF epilogue's engine drains block until
  the HWDGE queues are empty (verified bit-exact on HW).

No collectives. Rejected alternatives (all HW-measured): 8x
indirect_dma_start (no library tax but 1.41us/op NX-serial -> ~23.0us
total), single dma_gather on queue 0 (8.4ns/row serial on one pair ->
31.1us), DRAM->DRAM indirect (NRT_EXEC_UNIT_UNRECOVERABLE), multi-index
offset APs (HW reads one offset per partition), one-hot matmul
(compute-bound).
"""

import contextlib

import numpy as np

import concourse.bass as bass
from concourse import bacc, mybir
from concourse.bass_utils import run_bass_kernel_spmd
from concourse.library_config import mlp

N_CORES = 8
B, S = 2, 4096
VOCAB, DIM = 32000, 128
P = 128
TOKENS = B * S                      # 8192
TPC = TOKENS // N_CORES             # 1024 tokens per core
NQ = 4                              # SWDGE queues / Q7 core pairs used
RPQ = TPC // NQ                     # 256 rows per queue op
CBQ = RPQ // P                      # 2 col-blocks per op
IDX_COLS = RPQ // 16                # 16 int16 per idx partition


def build_nc():
    # Skip the Bass-constructor entry barrier (gates the first DMA behind
    # all engines' init); restore the method right after construction.
    orig_barrier = bass.Bass.all_engine_barrier
    bass.Bass.all_engine_barrier = lambda self, *a, **k: None
    try:
        nc = bacc.Bacc(
            None,
            target_bir_lowering=False,
            num_swdge_queues=NQ,
            dynamic_dma_scratch_size=65536,
        )
    finally:
        bass.Bass.all_engine_barrier = orig_barrier

    x = nc.dram_tensor("x", [P, IDX_COLS], mybir.dt.int16, kind="ExternalInput")
    w = nc.dram_tensor("weight", [VOCAB, DIM], mybir.dt.float32, kind="ExternalInput")
    out = nc.dram_tensor(
        "out", [P, NQ * CBQ, DIM], mybir.dt.float32, kind="ExternalOutput"
    )

    with contextlib.ExitStack() as ctx:
        idx_tile = ctx.enter_context(
            nc.sbuf_tensor("idx_tile", [P, IDX_COLS], mybir.dt.int16)
        )
        g = ctx.enter_context(
            nc.sbuf_tensor("g", [P, NQ * CBQ, DIM], mybir.dt.float32)
        )
        s_idx = ctx.enter_context(nc.semaphore("s_idx"))
        s_gs = [ctx.enter_context(nc.semaphore(f"s_g{q}")) for q in range(NQ)]
        s_out = ctx.enter_context(nc.semaphore("s_out"))

        # Start the ~8.7us Q7 library load first -- it is the critical path.
        nc.gpsimd.load_library(mlp)

        nc.scalar.dma_start(idx_tile[:], x[:]).then_inc(s_idx, 16)

        nc.gpsimd.wait_ge(s_idx, 16)
        # Queues 1..3 dispatch async to their pairs; queue 0 last (blocks).
        for q in [1, 2, 3, 0]:
            nc.gpsimd.dma_gather(
                g[:, q * CBQ : (q + 1) * CBQ, :],
                w[:],
                idx_tile[:],
                RPQ,
                RPQ,
                DIM,
                queue_num=q,
            ).then_inc(s_gs[q], 16)

        # All four gathers complete ~together (concurrent desc-gen + shared
        # SDMA drain), so one 512KB store beats four serialized block stores.
        for q in range(NQ):
            nc.sync.wait_ge(s_gs[q], 16)
        nc.sync.dma_start(out[:], g[:]).then_inc(s_out, 16)
    nc.compile()
    return nc


_NC_CACHE = None


def _pack_idx(idx_1d: np.ndarray) -> np.ndarray:
    """[TPC] ints -> [128, 16] int16 tile: partitions [32q, 32q+32) hold op
    q's 256 ids wrapped [16, 16] (id i at partition i%16, col i//16), x2."""
    tile = np.empty((P, IDX_COLS), dtype=np.int16)
    for q in range(NQ):
        wrapped = idx_1d[q * RPQ : (q + 1) * RPQ].astype(np.int16)
        wrapped = wrapped.reshape(IDX_COLS, 16).T        # [16, 16]
        tile[32 * q : 32 * (q + 1), :] = np.tile(wrapped, (2, 1))
    return np.ascontiguousarray(tile)


def kernel(x: np.ndarray, weight: np.ndarray, **run_kwargs):
    global _NC_CACHE
    if _NC_CACHE is None:
        _NC_CACHE = build_nc()
    nc = _NC_CACHE

    x_flat = np.asarray(x).reshape(-1)
    w = np.ascontiguousarray(np.asarray(weight, dtype=np.float32))

    in_maps = [
        {
            "x": _pack_idx(x_flat[c * TPC : (c + 1) * TPC]),
            "weight": w,
        }
        for c in range(N_CORES)
    ]
    res = run_bass_kernel_spmd(nc, in_maps, core_ids=list(range(N_CORES)), **run_kwargs)
    # out [128, 8, 128]: token q*256 + c*128 + p at [p, 2q+c, :] ->
    # transpose(1,0,2).reshape gives token-major [1024, 128].
    parts = [
        res.results[c]["out"].transpose(1, 0, 2).reshape(TPC, DIM)
        for c in range(N_CORES)
    ]
    full = np.concatenate(parts, axis=0).reshape(B, S, DIM)
    if run_kwargs:
        return full, res
    return full


# revision 13
# speedup vs baseline: 1.0359x; 1.0278x over previous
"""Embedding lookup kernel for Trainium2 (8 NeuronCores, SPMD).

Strategy: token-parallel gather (an embedding lookup IS a row gather:
out[b, s, :] = weight[x[b, s], :]).

- Flatten x [2, 4096] -> [8192] tokens; each of the 8 cores handles 1024
  contiguous tokens. Each core gets the FULL weight table in its DRAM.
- Per core (raw Bacc program, no Tile framework overhead; the Bass entry
  all-engine barrier is skipped):
    1. One HWDGE DMA issued from the Scalar engine (exits the framework
       preamble ~0.9us before Sync) loads the 1024 indices as [128, 8]
       int32 into SBUF (partition p holds tokens p*8 .. p*8+7).
       Completion is HBM-round-trip-bound (~2.3us).
    2. While that latency elapses, a dummy 16-row warmup indirect DMA
       (zero indices from a memset tile; no semaphore between them --
       memset runs on the same Q7 cluster, so program order suffices)
       runs on the Pool engine, absorbing the first-SWDGE-op ring-setup
       cost (~0.8us) so the real gathers run at steady state. 16 rows
       (8KB) keeps warmup SDMA traffic from delaying the idx DMA.
    3. 8 SWDGE indirect DMAs, one per token column j. One index per
       partition per op is a hard HW behavior: the DGE consumes ONE
       offset per partition and copies the dest partition's free run
       from it (verified: a [128, 8] offset AP yields one 4KB descriptor
       per partition reading idx[p,0] only; a strided dest that forces 8
       descs/partition scrambles data; DRAM->DRAM dest wedges the
       device, NRT_EXEC_UNIT_UNRECOVERABLE). Per-op cost is ~1.41us on
       the Pool engine (~1.03us fixed + ~0.5ns/row + ~0.31us post-op
       gap); the 512 KiB of gather traffic drains underneath.
    4. As each gather's completion fires on one accumulating semaphore,
       an HWDGE DMA stores that column block to DRAM
       out[:, j*128:(j+1)*128], overlapping the remaining gathers. No
       final completion wait: the NEFF epilogue's engine drains block
       until the HWDGE queues are empty (verified bit-exact on HW).
- dynamic_dma_scratch_size=65536 (4x default): the default SWDGE
  descriptor ring holds exactly 8x128 descs, so reclaim could stall the
  op train.
- out [128, 1024] f32 reshapes host-side to [1024, 128] (token p*8+j at
  partition p, col-block j). Host concatenates the 8 per-core outputs.

No collectives. Measured 23.2us exec (neuron-profile), bit-exact vs the
one-hot matmul reference. Rejected alternatives (all measured):
dma_gather batches 1024 rows in one op but its mlp Q7-library load costs
~8.7us in-kernel and its ucode runs at 8.4ns/row (31.1us total); one-hot
matmul is compute-bound (~23us+ at vocab/8 per core); SBUF-resident
table + ap_gather is Q7-throughput-bound plus the same library tax.
"""

import contextlib

import numpy as np

import concourse.bass as bass
from concourse import bacc, mybir
from concourse.bass_utils import run_bass_kernel_spmd

N_CORES = 8
B, S = 2, 4096
VOCAB, DIM = 32000, 128
P = 128
TOKENS = B * S                      # 8192
TPC = TOKENS // N_CORES             # 1024 tokens per core
TPP = TPC // P                      # 8 tokens per partition

WARM_ROWS = 16


def build_nc():
    # Skip the Bass-constructor entry barrier (gates the first DMA behind
    # all engines' init); restore the method right after construction.
    orig_barrier = bass.Bass.all_engine_barrier
    bass.Bass.all_engine_barrier = lambda self, *a, **k: None
    try:
        nc = bacc.Bacc(
            None, target_bir_lowering=False, dynamic_dma_scratch_size=65536
        )
    finally:
        bass.Bass.all_engine_barrier = orig_barrier

    x = nc.dram_tensor("x", [P, TPP], mybir.dt.int32, kind="ExternalInput")
    w = nc.dram_tensor("weight", [VOCAB, DIM], mybir.dt.float32, kind="ExternalInput")
    out = nc.dram_tensor("out", [P, TPC], mybir.dt.float32, kind="ExternalOutput")

    with contextlib.ExitStack() as ctx:
        idx_tile = ctx.enter_context(
            nc.sbuf_tensor("idx_tile", [P, TPP], mybir.dt.int32)
        )
        g = ctx.enter_context(nc.sbuf_tensor("g", [P, TPC], mybir.dt.float32))
        dummy_idx = ctx.enter_context(
            nc.sbuf_tensor("dummy_idx", [P, 1], mybir.dt.int32)
        )
        scratch = ctx.enter_context(
            nc.sbuf_tensor("scratch", [P, DIM], mybir.dt.float32)
        )
        s_idx = ctx.enter_context(nc.semaphore("s_idx"))
        s_warm = ctx.enter_context(nc.semaphore("s_warm"))
        s_g = ctx.enter_context(nc.semaphore("s_g"))
        s_out = ctx.enter_context(nc.semaphore("s_out"))

        nc.scalar.dma_start(idx_tile[:], x[:]).then_inc(s_idx, 16)

        # Warmup gather, hidden inside the idx-DMA latency window.
        nc.gpsimd.memset(dummy_idx[:], 0)
        nc.gpsimd.indirect_dma_start(
            out=scratch[0:WARM_ROWS, :],
            out_offset=None,
            in_=w[:],
            in_offset=bass.IndirectOffsetOnAxis(
                ap=dummy_idx[0:WARM_ROWS, :], axis=0
            ),
        ).then_inc(s_warm, 16)

        nc.gpsimd.wait_ge(s_idx, 16)
        for j in range(TPP):
            nc.gpsimd.indirect_dma_start(
                out=g[:, j * DIM : (j + 1) * DIM],
                out_offset=None,
                in_=w[:],
                in_offset=bass.IndirectOffsetOnAxis(ap=idx_tile[:, j : j + 1], axis=0),
            ).then_inc(s_g, 16)
        for j in range(TPP):
            nc.sync.wait_ge(s_g, 16 * (j + 1))
            nc.sync.dma_start(
                out[:, j * DIM : (j + 1) * DIM], g[:, j * DIM : (j + 1) * DIM]
            ).then_inc(s_out, 16)
    nc.compile()
    return nc


_NC_CACHE = None


def kernel(x: np.ndarray, weight: np.ndarray, **run_kwargs):
    global _NC_CACHE
    if _NC_CACHE is None:
        _NC_CACHE = build_nc()
    nc = _NC_CACHE

    x_flat = np.asarray(x).reshape(-1).astype(np.int32)
    w = np.ascontiguousarray(np.asarray(weight, dtype=np.float32))

    in_maps = [
        {
            "x": np.ascontiguousarray(x_flat[c * TPC : (c + 1) * TPC].reshape(P, TPP)),
            "weight": w,
        }
        for c in range(N_CORES)
    ]
    res = run_bass_kernel_spmd(nc, in_maps, core_ids=list(range(N_CORES)), **run_kwargs)
    # out [128, 1024] -> [1024, 128]: token p*TPP+j lives at [p, j*DIM:(j+1)*DIM]
    parts = [res.results[c]["out"].reshape(TPC, DIM) for c in range(N_CORES)]
    full = np.concatenate(parts, axis=0).reshape(B, S, DIM)
    if run_kwargs:
        return full, res
    return full


# revision 17
# speedup vs baseline: 1.0432x; 1.0070x over previous
"""Embedding lookup kernel for Trainium2 (8 NeuronCores, SPMD).

Strategy: token-parallel gather (an embedding lookup IS a row gather:
out[b, s, :] = weight[x[b, s], :]).

- Flatten x [2, 4096] -> [8192] tokens; each of the 8 cores handles 1024
  contiguous tokens. Each core gets the FULL weight table in its DRAM.
- Per core (raw Bacc program, no Tile framework overhead; the Bass entry
  all-engine barrier is skipped):
    1. One HWDGE DMA issued from the Scalar engine (exits the framework
       preamble ~0.9us before Sync) loads the 1024 indices as [128, 8]
       int32 into SBUF (partition p holds tokens p*8 .. p*8+7).
       Completion is HBM-round-trip-bound (~2.3us).
    2. While that latency elapses, a dummy 16-row warmup indirect DMA
       (zero indices from a memset tile; no semaphore between them --
       memset runs on the same Q7 cluster, so program order suffices)
       runs on the Pool engine, absorbing the first-SWDGE-op ring-setup
       cost (~0.8us) so the real gathers run at steady state. 16 rows
       (8KB) keeps warmup SDMA traffic from delaying the idx DMA.
    3. 8 SWDGE indirect DMAs, one per token column j. One index per
       partition per op is a hard HW behavior: the DGE consumes ONE
       offset per partition and copies the dest partition's free run
       from it (verified: a [128, 8] offset AP yields one 4KB descriptor
       per partition reading idx[p,0] only; a strided dest that forces 8
       descs/partition scrambles data; DRAM->DRAM dest wedges the
       device, NRT_EXEC_UNIT_UNRECOVERABLE). Per-op cost is ~1.41us on
       the Pool engine (~1.03us fixed + ~0.5ns/row + ~0.31us post-op
       gap); the 512 KiB of gather traffic drains underneath.
    4. As each gather's completion fires on one accumulating semaphore,
       an HWDGE DMA stores that column block to DRAM
       out[:, j*128:(j+1)*128], overlapping the remaining gathers. No
       final completion wait: the NEFF epilogue's engine drains block
       until the HWDGE queues are empty (verified bit-exact on HW).
- dynamic_dma_scratch_size=65536 (4x default): the default SWDGE
  descriptor ring holds exactly 8x128 descs, so reclaim could stall the
  op train.
- out [128, 1024] f32 reshapes host-side to [1024, 128] (token p*8+j at
  partition p, col-block j). Host concatenates the 8 per-core outputs.

No collectives. Measured 23.2us exec (neuron-profile), bit-exact vs the
one-hot matmul reference. Rejected alternatives (all measured):
dma_gather batches 1024 rows in one op but its mlp Q7-library load costs
~8.7us in-kernel and its ucode runs at 8.4ns/row (31.1us total); one-hot
matmul is compute-bound (~23us+ at vocab/8 per core); SBUF-resident
table + ap_gather is Q7-throughput-bound plus the same library tax.
"""

import contextlib

import numpy as np

import concourse.bass as bass
from concourse import bacc, mybir
from concourse.bass_utils import run_bass_kernel_spmd

N_CORES = 8
B, S = 2, 4096
VOCAB, DIM = 32000, 128
P = 128
TOKENS = B * S                      # 8192
TPC = TOKENS // N_CORES             # 1024 tokens per core
TPP = TPC // P                      # 8 tokens per partition
IDX_REP = 16                        # idx values replicated -> 512B/partition

WARM_ROWS = 16


def _indirect_gather_sp(gp, out_ap, in_ap, offset_ap):
    """indirect_dma_start clone (SBUF dest) with single_packet=True: each
    SDMA engine gets one 8-desc/4KB packet per op instead of per-descriptor
    packets, cutting ring context switches during the drain."""
    out_l = gp.lower_ap_dma(out_ap, for_indirect_dma=True)
    in_l = gp.lower_ap_dma(in_ap, for_indirect_dma=True)
    assert len(in_l) == 1 and len(out_l) == 1
    off_l = gp.lower_ap_dma(offset_ap)
    assert len(off_l) == 1
    in_l.append(off_l[0])

    ap_shape = in_ap.shape
    coef = 1
    for i in range(1, len(ap_shape)):
        coef *= ap_shape[i]
    in_l[0].dynamic_ap_info = mybir.DynamicAccessPatternInfo(
        c=0,
        actual_ap=out_ap.ap,
        indirect_dim_max_index=ap_shape[0],
        offset_expr=[
            mybir.DynamicAccessPatternOffsetExpr(
                coef=coef,
                aff_expr=mybir.DynamicAccessPatternOffsetExprAffExpr(
                    kind="IndirectArgId",
                    arg_id=1,
                ),
            )
        ],
    )
    return gp.add_instruction(
        mybir.InstDMACopy(
            name=gp.bass.get_next_instruction_name(),
            queue="qPoolDynamic",
            mode="Copy",
            ins=in_l,
            outs=out_l,
            oob_is_err=True,
            cce_op=mybir.AluOpType.bypass,
            single_packet=True,
        )
    )


def build_nc():
    # Skip the Bass-constructor entry barrier (gates the first DMA behind
    # all engines' init); restore the method right after construction.
    orig_barrier = bass.Bass.all_engine_barrier
    bass.Bass.all_engine_barrier = lambda self, *a, **k: None
    try:
        nc = bacc.Bacc(
            None, target_bir_lowering=False, dynamic_dma_scratch_size=65536
        )
    finally:
        bass.Bass.all_engine_barrier = orig_barrier

    x = nc.dram_tensor(
        "x", [P, TPP * IDX_REP], mybir.dt.int32, kind="ExternalInput"
    )
    w = nc.dram_tensor("weight", [VOCAB, DIM], mybir.dt.float32, kind="ExternalInput")
    out = nc.dram_tensor("out", [P, TPC], mybir.dt.float32, kind="ExternalOutput")

    with contextlib.ExitStack() as ctx:
        idx_tile = ctx.enter_context(
            nc.sbuf_tensor("idx_tile", [P, TPP * IDX_REP], mybir.dt.int32)
        )
        g = ctx.enter_context(nc.sbuf_tensor("g", [P, TPC], mybir.dt.float32))
        dummy_idx = ctx.enter_context(
            nc.sbuf_tensor("dummy_idx", [P, 1], mybir.dt.int32)
        )
        scratch = ctx.enter_context(
            nc.sbuf_tensor("scratch", [P, DIM], mybir.dt.float32)
        )
        s_idx = ctx.enter_context(nc.semaphore("s_idx"))
        s_warm = ctx.enter_context(nc.semaphore("s_warm"))
        s_g = ctx.enter_context(nc.semaphore("s_g"))
        s_out = ctx.enter_context(nc.semaphore("s_out"))

        nc.scalar.dma_start(idx_tile[:], x[:]).then_inc(s_idx, 16)

        # Warmup gather, hidden inside the idx-DMA latency window.
        nc.gpsimd.memset(dummy_idx[:], 0)
        _indirect_gather_sp(
            nc.gpsimd, scratch[0:WARM_ROWS, :], w[:], dummy_idx[0:WARM_ROWS, :]
        ).then_inc(s_warm, 16)

        nc.gpsimd.wait_ge(s_idx, 16)
        for j in range(TPP):
            _indirect_gather_sp(
                nc.gpsimd,
                g[:, j * DIM : (j + 1) * DIM],
                w[:],
                idx_tile[:, j * IDX_REP : j * IDX_REP + 1],
            ).then_inc(s_g, 16)
        for j in range(TPP):
            nc.sync.wait_ge(s_g, 16 * (j + 1))
            nc.sync.dma_start(
                out[:, j * DIM : (j + 1) * DIM], g[:, j * DIM : (j + 1) * DIM]
            ).then_inc(s_out, 16)
    nc.compile()
    return nc


_NC_CACHE = None


def kernel(x: np.ndarray, weight: np.ndarray, **run_kwargs):
    global _NC_CACHE
    if _NC_CACHE is None:
        _NC_CACHE = build_nc()
    nc = _NC_CACHE

    x_flat = np.asarray(x).reshape(-1).astype(np.int32)
    w = np.ascontiguousarray(np.asarray(weight, dtype=np.float32))

    in_maps = [
        {
            # [128, 8] -> [128, 128]: column 16j..16j+15 all hold token p*8+j,
            # giving 512B/partition DMA descriptors (32B/partition is the
            # known-slow sub-512B path).
            "x": np.ascontiguousarray(
                np.repeat(
                    x_flat[c * TPC : (c + 1) * TPC].reshape(P, TPP), IDX_REP, axis=1
                )
            ),
            "weight": w,
        }
        for c in range(N_CORES)
    ]
    res = run_bass_kernel_spmd(nc, in_maps, core_ids=list(range(N_CORES)), **run_kwargs)
    # out [128, 1024] -> [1024, 128]: token p*TPP+j lives at [p, j*DIM:(j+1)*DIM]
    parts = [res.results[c]["out"].reshape(TPC, DIM) for c in range(N_CORES)]
    full = np.concatenate(parts, axis=0).reshape(B, S, DIM)
    if run_kwargs:
        return full, res
    return full


# revision 21
# speedup vs baseline: 1.0466x; 1.0033x over previous
"""Embedding lookup kernel for Trainium2 (8 NeuronCores, SPMD).

Strategy: token-parallel gather (an embedding lookup IS a row gather:
out[b, s, :] = weight[x[b, s], :]).

- Flatten x [2, 4096] -> [8192] tokens; each of the 8 cores handles 1024
  contiguous tokens. Each core gets the FULL weight table in its DRAM.
- Per core (raw Bacc program, no Tile framework overhead; the Bass entry
  all-engine barrier is skipped):
    1. One HWDGE DMA issued from the Scalar engine (exits the framework
       preamble ~0.9us before Sync) loads the 1024 indices as [128, 8]
       int32 into SBUF (partition p holds tokens p*8 .. p*8+7).
       Completion is HBM-round-trip-bound (~2.3us).
    2. While that latency elapses, a dummy 16-row warmup indirect DMA
       (zero indices from a memset tile; no semaphore between them --
       memset runs on the same Q7 cluster, so program order suffices)
       runs on the Pool engine, absorbing the first-SWDGE-op ring-setup
       cost (~0.8us) so the real gathers run at steady state. 16 rows
       (8KB) keeps warmup SDMA traffic from delaying the idx DMA.
    3. 8 SWDGE indirect DMAs, one per token column j. One index per
       partition per op is a hard HW behavior: the DGE consumes ONE
       offset per partition and copies the dest partition's free run
       from it (verified: a [128, 8] offset AP yields one 4KB descriptor
       per partition reading idx[p,0] only; a strided dest that forces 8
       descs/partition scrambles data; DRAM->DRAM dest wedges the
       device, NRT_EXEC_UNIT_UNRECOVERABLE). Per-op cost is ~1.41us on
       the Pool engine (~1.03us fixed + ~0.5ns/row + ~0.31us post-op
       gap); the 512 KiB of gather traffic drains underneath.
    4. As each gather's completion fires on one accumulating semaphore,
       an HWDGE DMA stores that column block to DRAM
       out[:, j*128:(j+1)*128], overlapping the remaining gathers. No
       final completion wait: the NEFF epilogue's engine drains block
       until the HWDGE queues are empty (verified bit-exact on HW).
- dynamic_dma_scratch_size=65536 (4x default): the default SWDGE
  descriptor ring holds exactly 8x128 descs, so reclaim could stall the
  op train.
- out [128, 1024] f32 reshapes host-side to [1024, 128] (token p*8+j at
  partition p, col-block j). Host concatenates the 8 per-core outputs.

No collectives. Measured 23.2us exec (neuron-profile), bit-exact vs the
one-hot matmul reference. Rejected alternatives (all measured):
dma_gather batches 1024 rows in one op but its mlp Q7-library load costs
~8.7us in-kernel and its ucode runs at 8.4ns/row (31.1us total); one-hot
matmul is compute-bound (~23us+ at vocab/8 per core); SBUF-resident
table + ap_gather is Q7-throughput-bound plus the same library tax.
"""

import contextlib

import numpy as np

import concourse.bass as bass
from concourse import bacc, mybir
from concourse.bass_utils import run_bass_kernel_spmd

N_CORES = 8
B, S = 2, 4096
VOCAB, DIM = 32000, 128
P = 128
TOKENS = B * S                      # 8192
TPC = TOKENS // N_CORES             # 1024 tokens per core
TPP = TPC // P                      # 8 tokens per partition

WARM_ROWS = 16


def _indirect_gather_sp(gp, out_ap, in_ap, offset_ap):
    """indirect_dma_start clone (SBUF dest) with single_packet=True: each
    SDMA engine gets one 8-desc/4KB packet per op instead of per-descriptor
    packets, cutting ring context switches during the drain."""
    out_l = gp.lower_ap_dma(out_ap, for_indirect_dma=True)
    in_l = gp.lower_ap_dma(in_ap, for_indirect_dma=True)
    assert len(in_l) == 1 and len(out_l) == 1
    off_l = gp.lower_ap_dma(offset_ap)
    assert len(off_l) == 1
    in_l.append(off_l[0])

    ap_shape = in_ap.shape
    coef = 1
    for i in range(1, len(ap_shape)):
        coef *= ap_shape[i]
    in_l[0].dynamic_ap_info = mybir.DynamicAccessPatternInfo(
        c=0,
        actual_ap=out_ap.ap,
        indirect_dim_max_index=ap_shape[0],
        offset_expr=[
            mybir.DynamicAccessPatternOffsetExpr(
                coef=coef,
                aff_expr=mybir.DynamicAccessPatternOffsetExprAffExpr(
                    kind="IndirectArgId",
                    arg_id=1,
                ),
            )
        ],
    )
    return gp.add_instruction(
        mybir.InstDMACopy(
            name=gp.bass.get_next_instruction_name(),
            queue="qPoolDynamic",
            mode="Copy",
            ins=in_l,
            outs=out_l,
            oob_is_err=True,
            cce_op=mybir.AluOpType.bypass,
            single_packet=True,
        )
    )


def build_nc():
    # Skip the Bass-constructor entry barrier (gates the first DMA behind
    # all engines' init); restore the method right after construction.
    orig_barrier = bass.Bass.all_engine_barrier
    bass.Bass.all_engine_barrier = lambda self, *a, **k: None
    try:
        nc = bacc.Bacc(
            None, target_bir_lowering=False, dynamic_dma_scratch_size=65536
        )
    finally:
        bass.Bass.all_engine_barrier = orig_barrier

    x = nc.dram_tensor("x", [P, TPP], mybir.dt.int32, kind="ExternalInput")
    w = nc.dram_tensor("weight", [VOCAB, DIM], mybir.dt.float32, kind="ExternalInput")
    out = nc.dram_tensor("out", [P, TPC], mybir.dt.float32, kind="ExternalOutput")

    with contextlib.ExitStack() as ctx:
        idx_tile = ctx.enter_context(
            nc.sbuf_tensor("idx_tile", [P, TPP], mybir.dt.int32)
        )
        g = ctx.enter_context(nc.sbuf_tensor("g", [P, TPC], mybir.dt.float32))
        dummy_idx = ctx.enter_context(
            nc.sbuf_tensor("dummy_idx", [P, 1], mybir.dt.int32)
        )
        scratch = ctx.enter_context(
            nc.sbuf_tensor("scratch", [P, DIM], mybir.dt.float32)
        )
        s_idx = ctx.enter_context(nc.semaphore("s_idx"))
        s_warm = ctx.enter_context(nc.semaphore("s_warm"))
        s_g = ctx.enter_context(nc.semaphore("s_g"))
        s_out = ctx.enter_context(nc.semaphore("s_out"))

        nc.scalar.dma_start(idx_tile[:], x[:]).then_inc(s_idx, 16)

        # Warmup gather, hidden inside the idx-DMA latency window.
        nc.gpsimd.memset(dummy_idx[:], 0)
        _indirect_gather_sp(
            nc.gpsimd, scratch[0:WARM_ROWS, :], w[:], dummy_idx[0:WARM_ROWS, :]
        ).then_inc(s_warm, 16)

        nc.gpsimd.wait_ge(s_idx, 16)
        for j in range(TPP):
            _indirect_gather_sp(
                nc.gpsimd,
                g[:, j * DIM : (j + 1) * DIM],
                w[:],
                idx_tile[:, j : j + 1],
            ).then_inc(s_g, 16)
        for j in range(TPP):
            nc.sync.wait_ge(s_g, 16 * (j + 1))
            nc.sync.dma_start(
                out[:, j * DIM : (j + 1) * DIM], g[:, j * DIM : (j + 1) * DIM]
            ).then_inc(s_out, 16)
    nc.compile()
    return nc


_NC_CACHE = None


def kernel(x: np.ndarray, weight: np.ndarray, **run_kwargs):
    global _NC_CACHE
    if _NC_CACHE is None:
        _NC_CACHE = build_nc()
    nc = _NC_CACHE

    x_flat = np.asarray(x).reshape(-1).astype(np.int32)
    w = np.ascontiguousarray(np.asarray(weight, dtype=np.float32))

    in_maps = [
        {
            "x": np.ascontiguousarray(x_flat[c * TPC : (c + 1) * TPC].reshape(P, TPP)),
            "weight": w,
        }
        for c in range(N_CORES)
    ]
    res = run_bass_kernel_spmd(nc, in_maps, core_ids=list(range(N_CORES)), **run_kwargs)
    # out [128, 1024] -> [1024, 128]: token p*TPP+j lives at [p, j*DIM:(j+1)*DIM]
    parts = [res.results[c]["out"].reshape(TPC, DIM) for c in range(N_CORES)]
    full = np.concatenate(parts, axis=0).reshape(B, S, DIM)
    if run_kwargs:
        return full, res
    return full


# revision 27
# speedup vs baseline: 1.0479x; 1.0013x over previous
"""Embedding lookup kernel for Trainium2 (8 NeuronCores, SPMD).

Strategy: token-parallel gather (an embedding lookup IS a row gather:
out[b, s, :] = weight[x[b, s], :]).

- Flatten x [2, 4096] -> [8192] tokens; each of the 8 cores handles 1024
  contiguous tokens. Each core gets the FULL weight table in its DRAM.
- Per core (raw Bacc program, no Tile framework overhead; the Bass entry
  all-engine barrier is skipped):
    1. One HWDGE DMA issued from the Scalar engine (exits the framework
       preamble ~0.9us before Sync) loads the 1024 indices as [128, 8]
       int32 into SBUF (partition p holds tokens p*8 .. p*8+7).
       Completion is HBM-round-trip-bound (~2.3us).
    2. While that latency elapses, a dummy 16-row warmup indirect DMA
       (zero indices from a memset tile; no semaphore between them --
       memset runs on the same Q7 cluster, so program order suffices)
       runs on the Pool engine, absorbing the first-SWDGE-op ring-setup
       cost (~0.8us) so the real gathers run at steady state. 16 rows
       (8KB) keeps warmup SDMA traffic from delaying the idx DMA.
    3. 8 SWDGE indirect DMAs, one per token column j. One index per
       partition per op is a hard HW behavior: the DGE consumes ONE
       offset per partition and copies the dest partition's free run
       from it (verified: a [128, 8] offset AP yields one 4KB descriptor
       per partition reading idx[p,0] only; a strided dest that forces 8
       descs/partition scrambles data; DRAM->DRAM dest wedges the
       device, NRT_EXEC_UNIT_UNRECOVERABLE). Per-op cost is ~1.41us on
       the Pool engine (~1.03us fixed + ~0.5ns/row + ~0.31us post-op
       gap); the 512 KiB of gather traffic drains underneath. The ops
       are emitted with single_packet=True (one 8-desc/4KB packet per
       SDMA engine per op), which trimmed the last-op drain lag
       1.76us -> 1.57us.
    4. As each gather's completion fires on one accumulating semaphore,
       an HWDGE DMA stores that column block to DRAM
       out[:, j*128:(j+1)*128], overlapping the remaining gathers. No
       final completion wait: the NEFF epilogue's engine drains block
       until the HWDGE queues are empty (verified bit-exact on HW).
- dynamic_dma_scratch_size=65536 (4x default): the default SWDGE
  descriptor ring holds exactly 8x128 descs, so reclaim could stall the
  op train.
- out [128, 1024] f32 reshapes host-side to [1024, 128] (token p*8+j at
  partition p, col-block j). Host concatenates the 8 per-core outputs.

No collectives. Measured 23.2us exec (neuron-profile), bit-exact vs the
one-hot matmul reference. Rejected alternatives (all measured):
dma_gather batches 1024 rows in one op but its mlp Q7-library load costs
~8.7us in-kernel and its ucode runs at 8.4ns/row (31.1us total); one-hot
matmul is compute-bound (~23us+ at vocab/8 per core); SBUF-resident
table + ap_gather is Q7-throughput-bound plus the same library tax.
"""

import contextlib

import numpy as np

import concourse.bass as bass
from concourse import bacc, mybir
from concourse.bass_utils import run_bass_kernel_spmd

N_CORES = 8
B, S = 2, 4096
VOCAB, DIM = 32000, 128
P = 128
TOKENS = B * S                      # 8192
TPC = TOKENS // N_CORES             # 1024 tokens per core
TPP = TPC // P                      # 8 tokens per partition

WARM_ROWS = 16


def _indirect_gather_sp(gp, out_ap, in_ap, offset_ap):
    """indirect_dma_start clone (SBUF dest) with single_packet=True: each
    SDMA engine gets one 8-desc/4KB packet per op instead of per-descriptor
    packets, cutting ring context switches during the drain."""
    out_l = gp.lower_ap_dma(out_ap, for_indirect_dma=True)
    in_l = gp.lower_ap_dma(in_ap, for_indirect_dma=True)
    assert len(in_l) == 1 and len(out_l) == 1
    off_l = gp.lower_ap_dma(offset_ap)
    assert len(off_l) == 1
    in_l.append(off_l[0])

    ap_shape = in_ap.shape
    coef = 1
    for i in range(1, len(ap_shape)):
        coef *= ap_shape[i]
    in_l[0].dynamic_ap_info = mybir.DynamicAccessPatternInfo(
        c=0,
        actual_ap=out_ap.ap,
        indirect_dim_max_index=ap_shape[0],
        offset_expr=[
            mybir.DynamicAccessPatternOffsetExpr(
                coef=coef,
                aff_expr=mybir.DynamicAccessPatternOffsetExprAffExpr(
                    kind="IndirectArgId",
                    arg_id=1,
                ),
            )
        ],
    )
    return gp.add_instruction(
        mybir.InstDMACopy(
            name=gp.bass.get_next_instruction_name(),
            queue="qPoolDynamic",
            mode="Copy",
            ins=in_l,
            outs=out_l,
            oob_is_err=True,
            cce_op=mybir.AluOpType.bypass,
            single_packet=True,
        )
    )


def build_nc():
    # Skip the Bass-constructor entry barrier (gates the first DMA behind
    # all engines' init); restore the method right after construction.
    orig_barrier = bass.Bass.all_engine_barrier
    bass.Bass.all_engine_barrier = lambda self, *a, **k: None
    try:
        nc = bacc.Bacc(
            None, target_bir_lowering=False, dynamic_dma_scratch_size=65536
        )
    finally:
        bass.Bass.all_engine_barrier = orig_barrier

    x = nc.dram_tensor("x", [P, TPP], mybir.dt.int32, kind="ExternalInput")
    w = nc.dram_tensor("weight", [VOCAB, DIM], mybir.dt.float32, kind="ExternalInput")
    out = nc.dram_tensor("out", [P, TPC], mybir.dt.float32, kind="ExternalOutput")

    with contextlib.ExitStack() as ctx:
        idx_tile = ctx.enter_context(
            nc.sbuf_tensor("idx_tile", [P, TPP], mybir.dt.int32)
        )
        g = ctx.enter_context(nc.sbuf_tensor("g", [P, TPC], mybir.dt.float32))
        dummy_idx = ctx.enter_context(
            nc.sbuf_tensor("dummy_idx", [P, 1], mybir.dt.int32)
        )
        scratch = ctx.enter_context(
            nc.sbuf_tensor("scratch", [P, DIM], mybir.dt.float32)
        )
        s_idx = ctx.enter_context(nc.semaphore("s_idx"))
        s_warm = ctx.enter_context(nc.semaphore("s_warm"))
        s_g = ctx.enter_context(nc.semaphore("s_g"))
        s_out = ctx.enter_context(nc.semaphore("s_out"))

        nc.scalar.dma_start(idx_tile[:], x[:]).then_inc(s_idx, 16)

        # Warmup gather, hidden inside the idx-DMA latency window.
        nc.gpsimd.memset(dummy_idx[:], 0)
        _indirect_gather_sp(
            nc.gpsimd, scratch[0:WARM_ROWS, :], w[:], dummy_idx[0:WARM_ROWS, :]
        ).then_inc(s_warm, 16)

        nc.gpsimd.wait_ge(s_idx, 16)
        for j in range(TPP):
            _indirect_gather_sp(
                nc.gpsimd,
                g[:, j * DIM : (j + 1) * DIM],
                w[:],
                idx_tile[:, j : j + 1],
            ).then_inc(s_g, 16)
        for j in range(TPP):
            nc.sync.wait_ge(s_g, 16 * (j + 1))
            nc.sync.dma_start(
                out[:, j * DIM : (j + 1) * DIM], g[:, j * DIM : (j + 1) * DIM]
            ).then_inc(s_out, 16)
    nc.compile()
    return nc


_NC_CACHE = None


def kernel(x: np.ndarray, weight: np.ndarray, **run_kwargs):
    global _NC_CACHE
    if _NC_CACHE is None:
        _NC_CACHE = build_nc()
    nc = _NC_CACHE

    x_flat = np.asarray(x).reshape(-1).astype(np.int32)
    w = np.ascontiguousarray(np.asarray(weight, dtype=np.float32))

    in_maps = [
        {
            "x": np.ascontiguousarray(x_flat[c * TPC : (c + 1) * TPC].reshape(P, TPP)),
            "weight": w,
        }
        for c in range(N_CORES)
    ]
    res = run_bass_kernel_spmd(nc, in_maps, core_ids=list(range(N_CORES)), **run_kwargs)
    # out [128, 1024] -> [1024, 128]: token p*TPP+j lives at [p, j*DIM:(j+1)*DIM]
    parts = [res.results[c]["out"].reshape(TPC, DIM) for c in range(N_CORES)]
    full = np.concatenate(parts, axis=0).reshape(B, S, DIM)
    if run_kwargs:
        return full, res
    return full
